# revision 2
# baseline (speedup 1.0000x reference)
"""Direct 3x3 valid conv on 8 TRN2 cores — v3.

v2 (grouped weight reuse) + DMA restructure:
- One combined input tensor xin [128, 70, 128]: rows 0-5 = the six packed
  weight matrices, rows 6-69 = the 64 parity-packed row pairs. Per-partition
  contiguous, so a single band DMA covers weights AND the first row pairs.
- 3 input DMAs total (the trace showed DMA is packet-rate-limited at ~260ns
  per packet per engine with 8 packets per 128-partition descriptor, so
  many small bands were pure overhead): sync carries band 0 (weights +
  pairs 0-16, ~5.9KB/partition), gpsimd carries the two back bands.
- PE warmup bridges the pool barrier -> band-0 window so the HAM activity
  clock never resets (an idle PE gap re-throttles to 1.2GHz and costs
  ~4us of half-rate matmuls).
"""

import numpy as np
import ml_dtypes

import concourse.bass as bass
import concourse.mybir as mybir
import concourse.tile as tile
from concourse import bacc
from concourse.bass_utils import run_bass_kernel_spmd

N, C, H, W = 8, 64, 128, 128
K = 64
OH = OW = H - 2            # 126
P = H // 2                 # 64 input row pairs
JP = OH // 2               # 63 output row pairs
N_CORES = 8
J_CHUNK = 4                # row pairs per PSUM chunk: 4*126 = 504 <= 512
N_CHUNKS = (JP + J_CHUNK - 1) // J_CHUNK   # 16
MATS = [(u, dx) for u in range(2) for dx in range(3)]
N_WARM = 12
GROUPS = [[0, 1], [2, 3], [4, 5, 6, 7], [8, 9, 10, 11], [12, 13, 14, 15]]
XROWS = 6 + P              # 70: weights (6) + row pairs (64)
# Input bands, ALL serial on sync, smallest-first: the gating transfer is
# wt + pairs 0-4 (first group), later bands stream behind it while the PE
# consumes earlier chunks. Warmup matmuls bridge the ~4.4us from the pool
# barrier to band-0 arrival so the HAM activity ramp never resets.
BANDS_SYNC = [(0, 6), (6, 15), (15, 23), (23, 31), (31, 39),
              (39, 47), (47, 55), (55, 63), (63, 70)]
BANDS_GPS = []

BF16 = mybir.dt.bfloat16
NP_BF16 = ml_dtypes.bfloat16

_cache = {}


class _LightTileContext(tile.TileContext):
    """TileContext with a minimal end-of-kernel epilogue (see kernel.py)."""

    def _drain_and_barrier(self, tick_clock, wait_clock):
        nc = self.nc
        popped = nc._tile_sem_poison_stack.pop()
        assert popped is self._sem_poison
        d = nc.sync.drain()
        wait_clock.add_sem_waits(
            d.ins, tile.ScopedClock({None: tick_clock.global_clock})
        )


def _build_nc():
    nc = bacc.Bacc(None)
    xin = nc.dram_tensor("xin", [128, XROWS, W], BF16, kind="ExternalInput")
    out = nc.dram_tensor("out", [128, JP, OW], BF16, kind="ExternalOutput")

    with _LightTileContext(nc) as tc:
        with (
            tc.tile_pool(name="xpool", bufs=1) as xpool,
            tc.tile_pool(name="opool", bufs=4) as opool,
            tc.tile_pool(name="psum", bufs=8, space="PSUM") as psum,
        ):
            xin_sb = xpool.tile([128, XROWS, W], BF16)

            for b0, b1 in BANDS_SYNC:
                nc.sync.dma_start(xin_sb[:, b0:b1, :], xin[:, b0:b1, :])
            for b0, b1 in BANDS_GPS:
                nc.gpsimd.dma_start(xin_sb[:, b0:b1, :], xin[:, b0:b1, :])

            def wt_ap(mi):
                return xin_sb[:, mi, :]

            def mov_ap(j0, jn, u, dx):
                r = 6 + j0 + u
                return xin_sb[:, r : r + jn, dx : dx + OW]

            # PE warmup while band 0 loads (no producer dependency).
            ones = nc.const_aps.tensor(1.0, (128, 504), BF16)
            ones_w = nc.const_aps.tensor(1.0, (128, 128), BF16)

            def ps_tile(name):
                return psum.tile(
                    [128, J_CHUNK, OW], mybir.dt.float32, tag="ps", name=name
                )

            warm_ps = ps_tile("warm_ps")
            for wi in range(N_WARM):
                mm = nc.tensor.matmul(
                    warm_ps[:], ones_w, ones, start=True, stop=True,
                    skip_group_check=True,
                )
                if wi > 0:
                    mm.ins.ldweights = False

            for group in GROUPS:
                pss = {ci: ps_tile(f"ps_{ci}") for ci in group}
                for mi, (u, dx) in enumerate(MATS):
                    # snake: reversed on even mi so W5 (mi=5) runs the group
                    # in natural order and chunk 15 is the final matmul.
                    order = list(reversed(group)) if mi % 2 == 0 else list(group)
                    for k, ci in enumerate(order):
                        j0 = ci * J_CHUNK
                        jn = min(J_CHUNK, JP - j0)
                        mm = nc.tensor.matmul(
                            pss[ci][:, :jn, :],
                            wt_ap(mi),
                            mov_ap(j0, jn, u, dx),
                            start=(mi == 0),
                            stop=(mi == len(MATS) - 1),
                        )
                        if k > 0:
                            mm.ins.ldweights = False
                # Evict into one contiguous per-group buffer (casts alternate
                # vector/scalar), then a single group DMA — bigger packets
                # and fewer descriptors than per-chunk DMAs. The last group
                # splits off chunk 15 so the final (smallest) transfer posts
                # as late data arrives.
                jg = group[0] * J_CHUNK
                rows_g = sum(min(J_CHUNK, JP - ci * J_CHUNK) for ci in group)
                ob = opool.tile([128, rows_g, OW], BF16, tag="ob", name=f"ob{group[0]}")
                last_group = group[-1] == N_CHUNKS - 1
                oq = nc.gpsimd if (group[0] // 2) % 2 == 0 else nc.scalar
                for hi, ci in enumerate(group):
                    j0 = ci * J_CHUNK
                    jn = min(J_CHUNK, JP - j0)
                    r0 = j0 - jg
                    ps = pss[ci]
                    if last_group and ci == group[-1]:
                        # flush everything before the final chunk's cast, on
                        # sync — idle after the input bands, so the descriptor
                        # posts immediately
                        nc.sync.dma_start(
                            out[:, jg : jg + r0, :], ob[:, :r0, :]
                        )
                    if hi % 2 == 0:
                        nc.vector.tensor_copy(ob[:, r0 : r0 + jn, :], ps[:, :jn, :])
                    else:
                        nc.scalar.copy(ob[:, r0 : r0 + jn, :], ps[:, :jn, :])
                if last_group:
                    r0 = group[-1] * J_CHUNK - jg
                    nc.scalar.dma_start(
                        out[:, jg + r0 : jg + rows_g, :], ob[:, r0:rows_g, :]
                    )
                else:
                    oq.dma_start(out[:, jg : jg + rows_g, :], ob[:])

    nc.finalize()
    return nc


def _shard_inputs(x, filt):
    # wt[tau*64+c, u*3+dx, q*64+k] = filt[k, c, 2u+tau-q, dx] (0 if dy invalid)
    filt = np.asarray(filt, dtype=np.float32)
    wt = np.zeros((128, 6, 128), dtype=np.float32)
    for u in range(2):
        for dx in range(3):
            m = u * 3 + dx
            for tau in range(2):
                for q in range(2):
                    dy = 2 * u + tau - q
                    if 0 <= dy <= 2:
                        wt[tau * 64:(tau + 1) * 64, m, q * 64:(q + 1) * 64] = (
                            filt[:, :, dy, dx].T
                        )
    wt = wt.astype(NP_BF16)

    # xb[s, tau*64+c, j, w] = x[s, c, 2j+tau, w]
    xb = np.asarray(x, dtype=np.float32).astype(NP_BF16)
    xb = np.ascontiguousarray(
        xb.reshape(N, C, P, 2, W).transpose(0, 3, 1, 2, 4)
    ).reshape(N, 128, P, W)

    xin = np.empty((N, 128, XROWS, W), dtype=NP_BF16)
    xin[:, :, :6, :] = wt[None]
    xin[:, :, 6:, :] = xb
    return [{"xin": xin[s]} for s in range(N_CORES)]


def _gather(results):
    y = np.empty((N, K, OH, OW), dtype=np.float32)
    for s in range(N_CORES):
        o = np.asarray(results[s]["out"]).astype(np.float32)   # [(q,k), j, w]
        y[s] = o.reshape(2, K, JP, OW).transpose(1, 2, 0, 3).reshape(K, OH, OW)
    return y


def kernel(x, filt, **run_kwargs):
    if "nc" not in _cache:
        _cache["nc"] = _build_nc()
    in_maps = _shard_inputs(x, filt)
    res = run_bass_kernel_spmd(_cache["nc"], in_maps, list(range(N_CORES)), **run_kwargs)
    _cache["last_results"] = res
    return _gather(res.results)
